# revision 1
# baseline (speedup 1.0000x reference)
"""Binarized-CNN BasicBlock (2x conv3x3 256ch + train-mode BN + hardtanh +
residual) on 8 trn2 NeuronCores, data-parallel over the batch.

Key structure:
  - binarize(x) in {-1,+1} stored as fp8 -> conv = exact integer sums in
    fp32 PSUM via 18 accumulating DoubleRow matmuls (9 taps x 2 out-halves)
    over a zero-padded 30x30 spatial layout.
  - conv bias b1/b2 cancel under training-mode BN (shift invariance) and are
    never applied.
  - sign(hardtanh(bn(v))) == sign(v*scale + bias) so conv2's input needs only
    an affine threshold of conv1's raw output.
  - BN statistics span the full batch: per-core partial sum/sumsq are
    all-reduced across the 8 cores, split per channel-half so the first
    half's collective hides under the second half's conv.
  - Fully pipelined: conv1 starts while input chunks still stream in; BN
    sumsq accumulates during PSUM drains; conv2's residual add is fused into
    its drains; the first half's output pass hides under the second half.
"""

import numpy as np
import ml_dtypes

import concourse.bacc as bacc
import concourse.tile as tile
from concourse import mybir
from concourse.bass_utils import run_bass_kernel_spmd

# ---------------- problem constants (hardcoded) ----------------
N_CORES = 8
N_FULL, C, H, W = 128, 256, 28, 28
NPC = N_FULL // N_CORES          # 16 images per core
HP = WP = 30                     # padded frame
IMG = HP * WP                    # 900
MARG = 32                        # margin so shifted reads stay in-bounds
# Channel-plane-interleaved padded layout: per image, plane ci=0 then ci=1 at
# stride IMGB (a multiple of 16, as DoubleRow's ko step requires). Keeping
# each image's planes adjacent makes every matmul's byte footprint local to
# its image, so the Tile scheduler lets conv groups start as soon as their
# chunk is binarized instead of serializing on the whole input pass.
IMGB = 912                       # plane stride inside an image pair (57*16)
IMGP = 2 * IMGB                  # bytes per image (both planes)
BUF = MARG + NPC * IMGP + IMGP   # + one image of tail slack for AP views
SLAB = 450                       # matmul moving free dim (15 padded rows)
NSLABS = NPC * 2                 # 32 (2 slabs per image)
GROUP = 2                        # psum tiles per accumulation group: small
                                 # groups keep 4 in flight in the 8 PSUM
                                 # banks, so drain lag never idles the PE
                                 # (micro-idles make the HAM clock-gate
                                 # oscillate and halve matmul throughput)
NGROUPS = NSLABS // GROUP        # 16
NTOT = N_FULL * H * W            # 100352 samples per channel (full batch)
EPS = 1e-5
QI = 2                           # images per streaming chunk
NCHUNK = NPC // QI               # 8 chunks
GPC = QI * 2 // GROUP            # psum groups per image chunk (2)

BF16 = mybir.dt.bfloat16
FP8 = mybir.dt.float8e4
F32 = mybir.dt.float32
AF = mybir.ActivationFunctionType
ALU = mybir.AluOpType
NP_FP8 = mybir.dt.np(FP8)


def _interior(buf, ci, a, q):
    """[128, q, 28, 28] view of the valid pixels of plane ci of images
    a..a+q in a plane-interleaved padded [128, BUF] buffer."""
    base = MARG + ci * IMGB + a * IMGP
    v = buf[:, base: base + q * IMGP]
    v = v.rearrange("p (n b) -> p n b", b=IMGP)[:, :, :IMG]
    v = v.rearrange("p n (r c) -> p n r c", r=HP, c=WP)
    return v[:, :, 1:29, 1:29]


def _rhs(buf, s, off, doff):
    """DoubleRow rhs AP [128, 2, SLAB] for slab s shifted by tap offset."""
    n_img, h = divmod(s, 2)
    base = MARG + n_img * IMGP + h * SLAB + doff[off]
    v = buf[:, base: base + IMGP]
    return v.rearrange("p (k b) -> p k b", k=2)[:, :, :SLAB]


def _x_dram_ap(xd, a, q, mi):
    """DRAM AP for images a..a+q, channel tile mi -> [128, q, 784]."""
    sl = xd[a: a + q, mi * 128: (mi + 1) * 128, :, :]
    return sl.rearrange("n c h w -> c n (h w)")


def _build(use_inject=True):
    nc = bacc.Bacc(
        "TRN2",
        target_bir_lowering=False,
        debug=False,
        num_devices=N_CORES,
    )
    xd = nc.dram_tensor("x", [NPC, C, H, W], BF16, kind="ExternalInput")
    # DoubleRow lhsT layout: [ki=128, off=9, ko=2, o=256] fp8, channel = ko*128+ki
    w1d = nc.dram_tensor("w1s", [128, 9, 2, 256], FP8, kind="ExternalInput")
    w2d = nc.dram_tensor("w2s", [128, 9, 2, 256], FP8, kind="ExternalInput")
    bnd = nc.dram_tensor("bnp", [128, 8], F32, kind="ExternalInput")
    yd = nc.dram_tensor("y", [NPC, C, H, W], F32, kind="ExternalOutput")

    # tap offsets in the padded layout
    doff = [(dy - 1) * WP + (dx - 1) for dy in range(3) for dx in range(3)]

    with tile.TileContext(nc) as tc:
        with (
            tc.tile_pool(name="wp", bufs=1) as wp,
            tc.tile_pool(name="xsp", bufs=1) as xsp,
            tc.tile_pool(name="vp", bufs=1) as vp,
            tc.tile_pool(name="small", bufs=1) as small,
            tc.tile_pool(name="instage", bufs=3) as instage,
            tc.tile_pool(name="rstage", bufs=6) as rstage,
            tc.tile_pool(name="upool", bufs=4) as upool,
            tc.tile_pool(name="scrp", bufs=1) as scrp,
            tc.tile_pool(name="psum", bufs=8, space="PSUM") as psum,
            tc.tile_pool(name="dram", bufs=1, space="DRAM") as dram,
        ):
            # ---- weight & bn param tiles (DMAs emitted after the first x
            # chunks so conv1's critical path isn't queued behind them) ----
            wsb = {
                conv: wp.tile([128, 9, 2, 256], FP8, tag=f"w{conv}",
                              name=f"w{conv}")
                for conv in (1, 2)
            }
            bnp = small.tile([128, 8], F32, tag="bnp")
            eps_sb = small.tile([128, 1], F32, tag="eps")
            nc.vector.memset(eps_sb[:], EPS)

            # ---- sign-input buffers (zeroed: pads/margins must be 0) ----
            # plane-interleaved fp8 layout (see IMGB above): DoubleRow's rhs
            # AP [ki, 2, N] covers both channel planes of one image.
            # xs: conv1 input; xs2: conv2 input (separate so the conv2-input
            # sign pass can overlap conv1's tail matmuls).
            xs_all = xsp.tile([128, BUF], FP8, tag="xs")
            xs2_all = xsp.tile([128, BUF], FP8, tag="xs2")
            # zero via f32-bitcast views (4x fewer elements per lane); split
            # xs so the first chunks unblock their sign passes early. All on
            # DVE: GpSimd elementwise contends with DVE for the shared SBUF
            # port (exclusive lock) and degrades both ~10x when concurrent.
            CUT = MARG + 4 * IMGP
            nc.vector.memset(xs_all[:, :CUT].bitcast(F32), 0.0)
            nc.vector.memset(xs_all[:, CUT:].bitcast(F32), 0.0)
            nc.vector.memset(xs2_all[:].bitcast(F32), 0.0)
            # conv outputs stored COMPACT (valid pixels only, [128, 16*784]):
            # v[m] holds conv1's raw output, later overwritten with
            # t2 = conv2 + residual during conv2's drains.
            v = [
                vp.tile([128, NPC * 784], BF16, name=f"v{ci}", tag=f"v{ci}")
                for ci in range(2)
            ]
            # The residual is NOT kept resident: x is re-DMAed from HBM in
            # f32 chunks during conv2 (the DMA engines are idle there) and
            # added into the PSUM interior before draining, so the drain
            # accums directly give sum(t2).

            # ---- load x, binarize into padded layout (chunk-interleaved so
            # conv1 can start after the first chunk) ----
            for k in range(NCHUNK):
                a = k * QI
                for ci in range(2):
                    st = instage.tile([128, QI * 784], BF16, tag="xin")
                    nc.sync.dma_start(
                        out=st.rearrange("p (n f) -> p n f", n=QI),
                        in_=_x_dram_ap(xd, a, QI, ci),
                    )
                    nc.scalar.activation(
                        out=_interior(xs_all, ci, a, QI),
                        in_=st.rearrange("p (n r c) -> p n r c", r=28, c=28),
                        func=AF.Sign,
                    )
                if k == 0:
                    nc.sync.dma_start(out=wsb[1][:], in_=w1d[:])
            # w2 is first needed at conv2, bnp after the first AllReduce:
            # keep both off the x-chunk stream that paces conv1
            nc.sync.dma_start(out=bnp[:], in_=bnd[:])
            nc.sync.dma_start(out=wsb[2][:], in_=w2d[:])

            # ---- per-(conv, half) BN stat tiles ----
            sac = {}    # per-slab sums from drains       [128, NSLABS]
            qac = {}    # per-chunk sumsq from squares    [128, NCHUNK]
            for layer in (1, 2):
                for m in range(2):
                    sac[layer, m] = small.tile(
                        [128, NSLABS], F32, name=f"sac{layer}{m}",
                        tag=f"sac{layer}{m}")
                    qac[layer, m] = small.tile(
                        [128, NCHUNK], F32, name=f"qac{layer}{m}",
                        tag=f"qac{layer}{m}")

            def conv_half(idx, src_all, m, inject=None):
                """One output-channel half of a 3x3 conv. dst = v[m].
                idx==1: drains copy PSUM->v (alternating DVE/ACT) w/ sum
                accum. idx==2: the residual x chunk is re-DMAed from HBM,
                DVE adds it into the PSUM interior, then ACT drains t2 -> v
                with the accum giving sum(t2) directly. After each group, an
                ACT Square pass accumulates the chunk's sumsq. inject(g)
                adds ops after group g."""
                sacv = sac[idx, m]
                qacv = qac[idx, m]
                rst = None
                for g in range(NGROUPS):
                    kchunk, sub = divmod(g, GPC)
                    if idx == 2 and sub == 0:
                        rst = rstage.tile(
                            [128, QI * 784], BF16, name=f"rst{m}_{kchunk}",
                            tag="rst")
                        nc.sync.dma_start(
                            out=rst.rearrange("p (n f) -> p n f", n=QI),
                            in_=_x_dram_ap(xd, kchunk * QI, QI, m),
                        )
                    ps = [
                        psum.tile([128, SLAB], F32, name=f"c{idx}ps{m}_{g}_{i}",
                                  tag="ps")
                        for i in range(GROUP)
                    ]
                    for off in range(9):
                        lhsT = wsb[idx][:, off, :, m * 128: m * 128 + 128]
                        for s4 in range(GROUP):
                            s = g * GROUP + s4
                            nc.tensor.matmul(
                                ps[s4][:],
                                lhsT,
                                _rhs(src_all, s, off, doff),
                                start=(off == 0),
                                stop=(off == 8),
                                perf_mode=mybir.MatmulPerfMode.DoubleRow,
                            )
                    for s4 in range(GROUP):
                        s = g * GROUP + s4
                        n_img, h = divmod(s, 2)
                        # slab covers padded rows [15h, 15h+15) of image
                        # n_img; extract its valid 14x28 block compactly
                        src_ap = ps[s4].rearrange(
                            "p (r c) -> p r c", r=15, c=30
                        )[:, 1 - h: 15 - h, 1:29]
                        cb = n_img * 784 + h * 392
                        dst_ap = v[m][:, cb: cb + 392].rearrange(
                            "p (r c) -> p r c", r=14, c=28)
                        acc = sacv[:, s: s + 1]
                        if idx == 2:
                            rb = (n_img - 2 * kchunk) * 784 + h * 392
                            res_ap = rst[:, rb: rb + 392].rearrange(
                                "p (r c) -> p r c", r=14, c=28)
                            nc.vector.tensor_tensor(
                                out=src_ap, in0=src_ap, in1=res_ap,
                                op=ALU.add,
                            )
                            nc.scalar.activation(
                                out=dst_ap, in_=src_ap,
                                func=AF.Identity, accum_out=acc,
                            )
                        elif m == 0 or s4 % 2 == 0:
                            # conv1 m=0: all drains on DVE — ACT's FIFO is
                            # occupied by the DMA-paced input sign stream
                            # there, and drains queued behind it stall PSUM
                            nc.vector.tensor_scalar(
                                out=dst_ap, in0=src_ap,
                                scalar1=0.0, scalar2=None,
                                op0=ALU.add, op1=ALU.add, accum_out=acc,
                            )
                        else:
                            nc.scalar.activation(
                                out=dst_ap, in_=src_ap,
                                func=AF.Identity, accum_out=acc,
                            )
                    if sub == GPC - 1:
                        # image chunk complete for this half: fold sumsq in
                        ck = slice(kchunk * QI * 784, (kchunk + 1) * QI * 784)
                        scr = scrp.tile([128, QI * 784], BF16, tag="scr")
                        nc.scalar.activation(
                            out=scr[:],
                            in_=v[m][:, ck],
                            func=AF.Square,
                            accum_out=qacv[:, kchunk: kchunk + 1],
                        )
                        if inject is not None:
                            inject(kchunk)

            def start_allreduce(layer, m, after=None):
                """Reduce this half's stats and kick off its AllReduce.
                `after` (the previous AR's result tile) is folded in as a
                zero so the scheduler cannot hoist this AR's long-waiting
                stat DMA ahead of that result fetch in the engine FIFO
                (head-of-line blocking)."""
                stat = small.tile(
                    [128, 2], F32, name=f"st{layer}{m}", tag=f"stat{layer}{m}")
                if after is not None:
                    nc.vector.tensor_scalar_mul(stat[:, 0:1], after[:, 0:1], 0.0)
                nc.vector.reduce_sum(
                    out=stat[:, 0:1], in_=sac[layer, m][:],
                    axis=mybir.AxisListType.X,
                )
                nc.vector.reduce_sum(
                    out=stat[:, 1:2], in_=qac[layer, m][:],
                    axis=mybir.AxisListType.X,
                )
                # keep these tiny DMAs off the sync engine's HWDGE ring: a
                # long-waiting stats DMA there head-of-line blocks the AR
                # result fetch (and with it the whole downstream pass)
                in_b = dram.tile([128, 2], F32, tag=f"arin{layer}{m}")
                out_b = dram.tile([128, 2], F32, tag=f"arout{layer}{m}")
                nc.gpsimd.dma_start(out=in_b[:], in_=stat[:])
                nc.gpsimd.collective_compute(
                    "AllReduce",
                    ALU.add,
                    replica_groups=[list(range(N_CORES))],
                    ins=[in_b.opt()],
                    outs=[out_b.opt()],
                )
                red = small.tile([128, 2], F32, tag=f"red{layer}{m}")
                nc.gpsimd.dma_start(out=red[:], in_=out_b[:])
                return red

            def bn_coeffs(red, layer, m):
                """scale = gamma*rsqrt(var+eps); bias = beta - mean*scale."""
                name = f"{layer}{m}"
                mean = small.tile([128, 1], F32, tag=f"mean{name}")
                nc.vector.tensor_scalar_mul(mean[:], red[:, 0:1], 1.0 / NTOT)
                ex2 = small.tile([128, 1], F32, tag=f"ex2{name}")
                nc.vector.tensor_scalar_mul(ex2[:], red[:, 1:2], 1.0 / NTOT)
                var = small.tile([128, 1], F32, tag=f"var{name}")
                nc.vector.tensor_tensor(
                    out=var[:], in0=mean[:], in1=mean[:], op=ALU.mult
                )
                nc.vector.tensor_tensor(
                    out=var[:], in0=ex2[:], in1=var[:], op=ALU.subtract
                )
                std = small.tile([128, 1], F32, tag=f"std{name}")
                nc.scalar.activation(
                    out=std[:], in_=var[:], func=AF.Sqrt, bias=eps_sb[:]
                )
                inv = small.tile([128, 1], F32, tag=f"inv{name}")
                nc.vector.reciprocal(out=inv[:], in_=std[:])
                gcol = 4 * m if layer == 1 else 4 * m + 2
                bcol = gcol + 1
                sc = small.tile([128, 1], F32, tag=f"sc{name}")
                nc.vector.tensor_tensor(
                    out=sc[:], in0=inv[:], in1=bnp[:, gcol: gcol + 1],
                    op=ALU.mult,
                )
                bi = small.tile([128, 1], F32, tag=f"bi{name}")
                nc.vector.tensor_tensor(
                    out=bi[:], in0=mean[:], in1=sc[:], op=ALU.mult
                )
                nc.vector.tensor_tensor(
                    out=bi[:], in0=bnp[:, bcol: bcol + 1], in1=bi[:],
                    op=ALU.subtract,
                )
                return sc, bi

            def sign_chunks(m, a, q, sc, bi):
                """xs2 plane m interior <- sign(v[m]*sc + bi), images a..a+q."""
                nc.scalar.activation(
                    out=_interior(xs2_all, m, a, q),
                    in_=v[m][:, a * 784: (a + q) * 784].rearrange(
                        "p (n r c) -> p n r c", r=28, c=28),
                    func=AF.Sign,
                    bias=bi[:],
                    scale=sc[:],
                )

            def final_chunk(m, k, sc, bi):
                """y chunk <- clip(v[m]*sc + bi, -1, 1) (v holds t2)."""
                a = k * QI
                u = upool.tile([128, QI * 784], F32, tag="u")
                if k % 3 == 2:
                    # every third chunk fully on DVE to balance ACT
                    nc.vector.tensor_scalar(
                        out=u[:],
                        in0=v[m][:, a * 784: (a + QI) * 784],
                        scalar1=sc[:], scalar2=bi[:],
                        op0=ALU.mult, op1=ALU.add,
                    )
                else:
                    nc.scalar.activation(
                        out=u[:],
                        in_=v[m][:, a * 784: (a + QI) * 784],
                        func=AF.Identity, bias=bi[:], scale=sc[:],
                    )
                nc.vector.tensor_scalar(
                    out=u[:], in0=u[:],
                    scalar1=-1.0, scalar2=1.0,
                    op0=ALU.max, op1=ALU.min,
                )
                nc.sync.dma_start(
                    out=_x_dram_ap(yd, a, QI, m),
                    in_=u.rearrange("p (n f) -> p n f", n=QI),
                )

            # ================= conv1 m=0 =================
            conv_half(1, xs_all, 0)
            red10 = start_allreduce(1, 0)

            # ================= conv1 m=1 =================
            # inject coeffs + the full m=0 sign pass a few groups in, so the
            # ACT/DVE FIFOs never head-of-line block on the collective.
            coef1 = {}

            def inj_c1m1(g):
                if g == 2:
                    coef1[0] = bn_coeffs(red10, 1, 0)
                    sign_chunks(0, 0, NPC, *coef1[0])

            conv_half(1, xs_all, 1, inject=inj_c1m1 if use_inject else None)
            red11 = start_allreduce(1, 1, after=red10)
            if not use_inject:
                coef1[0] = bn_coeffs(red10, 1, 0)
                sign_chunks(0, 0, NPC, *coef1[0])
            coef1[1] = bn_coeffs(red11, 1, 1)
            # prime two chunks of the m=1 sign so conv2 can start; the rest
            # stream in during conv2 m=0 with 2-group lookahead.
            sign_chunks(1, 0, QI, *coef1[1])
            sign_chunks(1, QI, QI, *coef1[1])
            if not use_inject:
                for k in range(2, NCHUNK):
                    sign_chunks(1, k * QI, QI, *coef1[1])

            # ================= conv2 m=0 =================
            def inj_c2m0(g):
                if g + 2 < NCHUNK:
                    sign_chunks(1, (g + 2) * QI, QI, *coef1[1])

            conv_half(2, xs2_all, 0, inject=inj_c2m0 if use_inject else None)
            red20 = start_allreduce(2, 0, after=red11)

            # ================= conv2 m=1 =================
            # hide the m=0 output pass (affine+clip+store) under these MMs
            coef2 = {}

            def inj_c2m1(g):
                if g == 2:
                    coef2[0] = bn_coeffs(red20, 2, 0)
                if 2 <= g <= 5:
                    final_chunk(0, 2 * (g - 2), *coef2[0])
                    final_chunk(0, 2 * (g - 2) + 1, *coef2[0])

            conv_half(2, xs2_all, 1, inject=inj_c2m1 if use_inject else None)
            red21 = start_allreduce(2, 1, after=red20)
            if not use_inject:
                coef2[0] = bn_coeffs(red20, 2, 0)
                for k in range(NCHUNK):
                    final_chunk(0, k, *coef2[0])
            sc21, bi21 = bn_coeffs(red21, 2, 1)
            for k in range(NCHUNK):
                final_chunk(1, k, sc21, bi21)

    nc.compile()
    # pass-ordering bug in this bacc vintage: late compile passes can leave
    # >1 sync wait on an instruction (HW cap); one more split pass fixes it
    nc.generate_event_semaphores()
    return nc


_NC_CACHE = None
_RUNNER = None


def _get_nc():
    global _NC_CACHE
    if _NC_CACHE is None:
        _NC_CACHE = _build()
    return _NC_CACHE


def _make_runner(nc):
    """Persistent jitted shard_map over 8 cores (mirrors
    bass2jax.run_bass_via_pjrt but cached, so repeat calls skip retracing)."""
    import jax
    import jax.core
    from jax.sharding import Mesh, PartitionSpec
    from jax.experimental.shard_map import shard_map
    from concourse import bass2jax, mybir as mb

    bass2jax.install_neuronx_cc_hook()
    partition_name = (
        nc.partition_id_tensor.name if nc.partition_id_tensor else None
    )
    in_names, out_names, out_avals, zero_outs = [], [], [], []
    for alloc in nc.m.functions[0].allocations:
        if not isinstance(alloc, mb.MemoryLocationSet):
            continue
        name = alloc.memorylocations[0].name
        if alloc.kind == "ExternalInput":
            if name != partition_name:
                in_names.append(name)
        elif alloc.kind == "ExternalOutput":
            shape = tuple(alloc.tensor_shape)
            dtype = mb.dt.np(alloc.dtype)
            out_names.append(name)
            out_avals.append(jax.core.ShapedArray(shape, dtype))
            zero_outs.append(np.zeros(shape, dtype))
    n_params = len(in_names)
    n_outs = len(out_avals)
    all_in_names = list(in_names) + list(out_names)
    if partition_name is not None:
        all_in_names.append(partition_name)
    donate = tuple(range(n_params, n_params + n_outs))

    def _body(*args):
        operands = list(args)
        if partition_name is not None:
            operands.append(bass2jax.partition_id_tensor())
        outs = bass2jax._bass_exec_p.bind(
            *operands,
            out_avals=tuple(out_avals),
            in_names=tuple(all_in_names),
            out_names=tuple(out_names),
            lowering_input_output_aliases=(),
            sim_require_finite=True,
            sim_require_nnan=True,
            nc=nc,
        )
        return tuple(outs)

    devices = jax.devices()[:N_CORES]
    mesh = Mesh(np.asarray(devices), ("core",))
    in_specs = (PartitionSpec("core"),) * (n_params + n_outs)
    out_specs = (PartitionSpec("core"),) * n_outs
    sharded = jax.jit(
        shard_map(
            _body, mesh=mesh, in_specs=in_specs, out_specs=out_specs,
            check_rep=False,
        ),
        donate_argnums=donate,
        keep_unused=True,
    )
    return sharded, in_names, out_names, zero_outs


def _get_runner():
    global _RUNNER
    if _RUNNER is None:
        _RUNNER = _make_runner(_get_nc())
    return _RUNNER


def _prep_x(x):
    """x -> bf16 (halves the input DMA bytes; the +-0.4% residual rounding
    is far inside the 2e-2 gate)."""
    return np.asarray(x, dtype=np.float32).astype(ml_dtypes.bfloat16)


def _prep_weights(w):
    """[O=256,I=256,3,3] f32 -> sign-binarized DoubleRow lhsT layout
    [ki=128, off=9, ko=2, o=256] fp8: [ki,off,ko,o] = sign(w[o, ko*128+ki, off])."""
    ws = np.sign(np.asarray(w, dtype=np.float32))
    # [o, ko, ki, off] -> [ki, off, ko, o]
    ws = ws.reshape(256, 2, 128, 9).transpose(2, 3, 1, 0)
    return np.ascontiguousarray(ws.astype(NP_FP8))


def kernel(x, w1, b1, g1, be1, w2, b2, g2, be2):
    x = _prep_x(x)
    w1s = _prep_weights(w1)
    w2s = _prep_weights(w2)
    # per-partition BN params: [128, 8] cols = (g1,be1,g2,be2) for m=0, then m=1
    bnp = np.stack(
        [
            np.asarray(g1, np.float32).reshape(2, 128),
            np.asarray(be1, np.float32).reshape(2, 128),
            np.asarray(g2, np.float32).reshape(2, 128),
            np.asarray(be2, np.float32).reshape(2, 128),
        ],
        axis=-1,
    )  # [2, 128, 4]
    bnp = np.ascontiguousarray(bnp.transpose(1, 0, 2).reshape(128, 8))

    sharded, in_names, out_names, zero_outs = _get_runner()
    per_core = {
        "x": x.reshape(N_CORES * NPC, C, H, W),
        "w1s": np.concatenate([w1s] * N_CORES, axis=0),
        "w2s": np.concatenate([w2s] * N_CORES, axis=0),
        "bnp": np.concatenate([bnp] * N_CORES, axis=0),
    }
    concat_in = [per_core[name] for name in in_names]
    concat_zeros = [
        np.zeros((N_CORES * z.shape[0], *z.shape[1:]), z.dtype)
        for z in zero_outs
    ]
    out_arrs = sharded(*concat_in, *concat_zeros)
    yi = out_names.index("y")
    return np.asarray(out_arrs[yi]).reshape(N_FULL, C, H, W)

